# revision 1
# baseline (speedup 1.0000x reference)
"""ChebyshevKANLayer on 8 Trainium2 NeuronCores.

y = silu(x) @ Wb + sum_d (x * T_d(xs)) @ Wc[:, :, d]
  xs = per-row rescale of x to [-1, 1]; T_d = Chebyshev polynomials.

Sharding: data-parallel over the batch dim (4096 -> 8 x 512 rows).
Weights replicated (shipped as bf16 to halve the dominant DMA traffic).
No collectives; the host concatenates the shards.

Per-core structure (measured rates: DMA ~326 GB/s, bf16 matmul ~104
ns per [128x128]x[128x512], fp32r ~123 ns):
  - phase A (emitted first so PE/DMA start immediately): silu path --
    sigmoid on ACT, the multiply on gpsimd (writing bf16), 64 matmuls
    into the 8 PSUM accumulators.
  - stats (overlaps phase A): row min/max on DVE from the natural
    shard, tiny affine ops, a 32x32 stream transpose + strided
    SBUF-SBUF DMA gather to form [1, 512] stat rows, then
    gpsimd.partition_broadcast -> [128, 512] broadcast tiles. No PE,
    no PSUM.
  - phase B: per contraction tile, u = 2*xs on DVE, Chebyshev
    recurrence on G_d = x*T_d in fp32 on DVE, per-degree bf16 casts on
    gpsimd, 64 bf16 matmuls.
  - epilogue: PSUM -> SBUF copies (DVE) + output DMA.
"""

import numpy as np

from concourse import bacc, masks, mybir, tile
from concourse.bass_utils import run_bass_kernel_spmd

B, IN, OUT, DEG = 4096, 1024, 1024, 8
NCORES = 8
BS = B // NCORES  # 512 rows per core
KT = IN // 128  # 8 contraction tiles
NB = BS // 128  # 4 batch tiles per core
NO = OUT // 512  # 2 output column tiles

F32 = mybir.dt.float32
BF16 = mybir.dt.bfloat16
ALU = mybir.AluOpType
AF = mybir.ActivationFunctionType
AX = mybir.AxisListType


def _build_kernel(tc, out, xt, xtb, xn, wb, wc, repeat=1):
    nc = tc.nc
    from contextlib import ExitStack

    octx = ExitStack()
    const_pool = octx.enter_context(tc.tile_pool(name="const", bufs=1))
    ident = const_pool.tile([128, 128], F32)
    masks.make_identity(nc, ident[:])
    ones = const_pool.tile([1, 128], F32)
    nc.vector.memset(ones[:], 1.0)
    sb = const_pool.tile([128, BS], F32)  # broadcast of 2*s per column
    tb = const_pool.tile([128, BS], F32)  # broadcast of 2*t per column
    s_row = const_pool.tile([1, BS], F32)
    t_row = const_pool.tile([1, BS], F32)

    with (
        tc.tile_pool(name="psum_acc", bufs=1, space="PSUM") as pacc,
        tc.tile_pool(name="w", bufs=2) as wpool,
        tc.tile_pool(name="g", bufs=2) as gpool,
        tc.tile_pool(name="gb", bufs=2) as gbpool,
        tc.tile_pool(name="xtp", bufs=1) as xtpool,
        tc.tile_pool(name="silu", bufs=2) as slpool,
        tc.tile_pool(name="u", bufs=2) as upool,
        tc.tile_pool(name="o", bufs=2) as opool,
        tc.tile_pool(name="stats", bufs=2) as spool,
    ):
        po = [
            [
                pacc.tile([128, 512], F32, tag=f"po{t}{j}", name=f"po{t}{j}")
                for j in range(NO)
            ]
            for t in range(NB)
        ]
        for rep in range(repeat):
            first = rep == 0
            # --- stats: row min/max -> u = 2*xs = x*s2 + t2 broadcast tiles.
            # The tiny PE-transpose / ones-matmul PSUM outputs alias into the
            # po accumulator banks: the PE runs them (in program order)
            # before the first accumulating matmul, whose start=True reset
            # wipes the scratch values.
            if first:
                for t in range(NB):
                    xnt = spool.tile([128, IN], F32, tag="xnt", name="xnt")
                    nc.sync.dma_start(out=xnt[:], in_=xn[t * 128 : (t + 1) * 128, :])
                    mx = spool.tile([128, 1], F32, tag="mx", name="mx")
                    mn = spool.tile([128, 1], F32, tag="mn", name="mn")
                    nc.vector.tensor_reduce(mx[:], xnt[:], axis=AX.X, op=ALU.max)
                    nc.vector.tensor_reduce(mn[:], xnt[:], axis=AX.X, op=ALU.min)
                    d = spool.tile([128, 1], F32, tag="d", name="d")
                    nc.vector.tensor_tensor(d[:], mx[:], mn[:], ALU.subtract)
                    r = spool.tile([128, 1], F32, tag="r", name="r")
                    nc.vector.reciprocal(r[:], d[:])
                    sc = spool.tile([128, 1], F32, tag="sc", name="sc")
                    nc.vector.tensor_scalar(sc[:], r[:], 4.0, None, ALU.mult)
                    tmp = spool.tile([128, 1], F32, tag="tmp", name="tmp")
                    nc.vector.tensor_tensor(tmp[:], mn[:], sc[:], ALU.mult)
                    tcn = spool.tile([128, 1], F32, tag="tcn", name="tcn")
                    nc.vector.tensor_scalar(
                        tcn[:], tmp[:], -1.0, -2.0, ALU.mult, ALU.add
                    )
                    tsl = slice(t * 128, (t + 1) * 128)
                    nc.tensor.transpose(po[0][0][0:1, tsl], sc[:], ident[:])
                    nc.vector.tensor_copy(s_row[0:1, tsl], po[0][0][0:1, tsl])
                    nc.tensor.transpose(po[0][1][0:1, tsl], tcn[:], ident[:])
                    nc.vector.tensor_copy(t_row[0:1, tsl], po[0][1][0:1, tsl])
                # broadcast the stat rows across all 128 partitions
                nc.tensor.matmul(
                    po[1][0][:], lhsT=ones[:], rhs=s_row[:], start=True, stop=True
                )
                nc.vector.tensor_copy(sb[:], po[1][0][:])
                nc.tensor.matmul(
                    po[1][1][:], lhsT=ones[:], rhs=t_row[:], start=True, stop=True
                )
                nc.vector.tensor_copy(tb[:], po[1][1][:])

            # --- phase A: silu path (independent of row stats) ---
            xtts = []
            xbts = []
            for k in range(KT):
                ksl = slice(k * 128, (k + 1) * 128)
                xtt = xtpool.tile([128, BS], F32, tag=f"xtt{k}", name=f"xtt{k}")
                xtts.append(xtt)
                nc.sync.dma_start(out=xtt[:], in_=xt[ksl, :])
                xbt = xtpool.tile([128, BS], BF16, tag=f"xbt{k}", name=f"xbt{k}")
                xbts.append(xbt)
                nc.sync.dma_start(out=xbt[:], in_=xtb[ksl, :])
                wbt = wpool.tile([128, OUT], BF16, tag="wbt", name="wbt")
                nc.sync.dma_start(out=wbt[:], in_=wb[ksl, :])
                sl = slpool.tile([128, BS], BF16, tag="sl", name="sl")
                sigt = slpool.tile([128, BS], BF16, tag="sigt", name="sigt")
                # silu = x*sigmoid(x); bf16 operands let the TT run in 2x mode
                nc.scalar.activation(sigt[:], xtt[:], AF.Sigmoid)
                nc.vector.tensor_tensor(sl[:], sigt[:], xbt[:], ALU.mult)
                for t in range(NB):
                    lhs = sl[:, t * 128 : (t + 1) * 128]
                    for j in range(NO):
                        rhs = wbt[:, j * 512 : (j + 1) * 512]
                        nc.tensor.matmul(
                            po[t][j][:],
                            lhsT=lhs,
                            rhs=rhs,
                            start=(k == 0),
                            stop=False,
                        )

            # --- phase B: chebyshev paths ---
            # Everything on DVE + ACT: in-context gpsimd ops measured ~2.5us
            # each (sem-wait + software dispatch), so the whole chain, the
            # cheap 312ns bf16 casts, and the silu multiply stay on DVE.
            GP_MULTS = ()  # chain mults placed on gpsimd (none)
            for k in range(KT):
                ksl = slice(k * 128, (k + 1) * 128)
                xtt = xtts[k]
                xbt = xbts[k]
                wall = wpool.tile([128, DEG * OUT], BF16, tag="wall", name="wall")
                for dg in range(DEG):
                    nc.sync.dma_start(
                        out=wall[:, dg * OUT : (dg + 1) * OUT], in_=wc[dg, ksl, :]
                    )
                gall = gpool.tile([128, (DEG - 1) * BS], F32, tag="gall", name="gall")
                gball = gbpool.tile(
                    [128, (DEG - 1) * BS], BF16, tag="gball", name="gball"
                )

                def Gs(i):
                    # fp32 recurrence slots G_1..G_7
                    return gall[:, (i - 1) * BS : i * BS]

                def Gb(i):
                    # bf16 matmul operand slots G_1..G_7
                    return gball[:, (i - 1) * BS : i * BS]

                ut = upool.tile([128, BS], F32, tag="ut", name="ut")
                nc.vector.tensor_tensor(ut[:], xtt[:], sb[:], ALU.mult)
                nc.vector.tensor_tensor(ut[:], ut[:], tb[:], ALU.add)
                # G_1 = x * xs = (x * 0.5) * u
                nc.vector.scalar_tensor_tensor(
                    Gs(1), in0=xtt[:], scalar=0.5, in1=ut[:], op0=ALU.mult, op1=ALU.mult
                )
                nc.scalar.activation(Gb(1), Gs(1), AF.Copy)
                for dg in range(2, DEG):
                    tmpd = upool.tile([128, BS], F32, tag=f"tmpd{dg}", name="tmpd")
                    meng = nc.gpsimd if dg in GP_MULTS else nc.vector
                    meng.tensor_tensor(tmpd[:], ut[:], Gs(dg - 1), ALU.mult)
                    prev2 = xtt[:] if dg == 2 else Gs(dg - 2)
                    nc.vector.tensor_tensor(Gs(dg), tmpd[:], prev2, ALU.subtract)
                    nc.scalar.activation(Gb(dg), Gs(dg), AF.Copy)

                gstat = [xbt] + [Gb(i) for i in range(1, DEG)]
                for t in range(NB):
                    for m in range(DEG):
                        lhs = gstat[m][:, t * 128 : (t + 1) * 128]
                        for j in range(NO):
                            rhs = wall[:, m * OUT + j * 512 : m * OUT + (j + 1) * 512]
                            nc.tensor.matmul(
                                po[t][j][:],
                                lhsT=lhs,
                                rhs=rhs,
                                start=False,
                                stop=(k == KT - 1 and m == DEG - 1),
                            )
        for t in range(NB):
            for j in range(NO):
                ot = opool.tile([128, 512], F32, tag="ot", name="ot")
                nc.scalar.activation(ot[:], po[t][j][:], AF.Copy)
                nc.sync.dma_start(
                    out=out[t * 128 : (t + 1) * 128, j * 512 : (j + 1) * 512],
                    in_=ot[:],
                )
    octx.close()


_NC_CACHE = {}


def build_nc(repeat=1):
    if repeat in _NC_CACHE:
        return _NC_CACHE[repeat]
    nc = bacc.Bacc(
        "TRN2", target_bir_lowering=False, debug=False, num_devices=NCORES
    )
    xt = nc.dram_tensor("xt", [IN, BS], F32, kind="ExternalInput").ap()
    xtb = nc.dram_tensor("xtb", [IN, BS], BF16, kind="ExternalInput").ap()
    xn = nc.dram_tensor("xn", [BS, IN], F32, kind="ExternalInput").ap()
    wb = nc.dram_tensor("wb", [IN, OUT], BF16, kind="ExternalInput").ap()
    wc = nc.dram_tensor("wc", [DEG, IN, OUT], BF16, kind="ExternalInput").ap()
    out = nc.dram_tensor("out", [BS, OUT], F32, kind="ExternalOutput").ap()
    with tile.TileContext(nc) as tc:
        _build_kernel(tc, out, xt, xtb, xn, wb, wc, repeat=repeat)
    nc.compile()
    _NC_CACHE[repeat] = nc
    return nc


def make_in_maps(x, base_weight, cheb_weight):
    import ml_dtypes

    x = np.ascontiguousarray(np.asarray(x, dtype=np.float32))
    wb = np.asarray(base_weight, dtype=np.float32).astype(ml_dtypes.bfloat16)
    wc = np.ascontiguousarray(
        np.asarray(cheb_weight, dtype=np.float32)
        .transpose(2, 0, 1)
        .astype(ml_dtypes.bfloat16)
    )
    in_maps = []
    for c in range(NCORES):
        shard = x[c * BS : (c + 1) * BS]
        shard_t = np.ascontiguousarray(shard.T)
        in_maps.append(
            {
                "xt": shard_t,
                "xtb": shard_t.astype(ml_dtypes.bfloat16),
                "xn": shard,
                "wb": wb,
                "wc": wc,
            }
        )
    return in_maps


def kernel(x, base_weight, cheb_weight, degree=DEG, **_):
    assert int(degree) == DEG
    nc = build_nc()
    in_maps = make_in_maps(x, base_weight, cheb_weight)
    res = run_bass_kernel_spmd(nc, in_maps, list(range(NCORES)))
    return np.concatenate([r["out"] for r in res.results], axis=0)



# revision 8
# speedup vs baseline: 1.0187x; 1.0187x over previous
"""ChebyshevKANLayer on 8 Trainium2 NeuronCores.

y = silu(x) @ Wb + sum_d (x * T_d(xs)) @ Wc[:, :, d]
  xs = per-row rescale of x to [-1, 1]; T_d = Chebyshev polynomials.

Sharding: data-parallel over the batch dim (4096 -> 8 x 512 rows),
weights replicated; no collectives, host concatenates the shards.

All matmul operands are fp16 (1 PE cycle/row, same rate as bf16 but
8x finer mantissa: end-to-end rel err ~4e-3 vs 2e-2 budget). The
Chebyshev recurrence runs on DVE in fp16 storage / fp32 ALU, which
enables the DVE 2x_1p mode (327ns per [128,512] op) and kills all
operand-cast traffic. Host packs weights as [wb | wc_d0 | .. | wc_d7]
so each contraction tile streams with two DMAs (wA: silu+d0 columns,
wB: d1..d7 columns).

Schedule (per core, cost-model-driven):
  - ~25 tiny dummy matmuls warm the PE P-state while the first DMAs
    land (PE ramps to 2.4GHz after 3us of continuous execution).
  - phase 1 (stats-independent): per k, d0 matmuls (lhsT = raw x f16)
    and silu matmuls (lhsT = ACT-native Silu of x). Meanwhile row
    min/max stats run on DVE+gpsimd from the natural-layout copy,
    tiny strided DMAs gather per-row scale/offset into [1,512] rows,
    and gpsimd.partition_broadcast forms the [128,512] u-coefficient
    tiles -- no PE, no PSUM involvement.
  - phase 2: per k, the fp16 G-chain (G_d = u*G_{d-1} - G_{d-2}) on
    DVE, then 56 accumulating matmuls; weight DMA (5.1us) and chain
    (5.2us) both fit inside the 11.9us PE window, pipelined one k
    ahead.
  - epilogue: the last k's matmuls run bank-major so each PSUM bank
    stops early and drains (ACT copy to f16 + DMA) under the
    remaining matmuls.
"""

import numpy as np

from concourse import bacc, mybir, tile
from concourse.bass_utils import run_bass_kernel_spmd

B, IN, OUT, DEG = 4096, 1024, 1024, 8
NCORES = 8
BS = B // NCORES  # 512 rows per core
KT = IN // 128  # 8 contraction tiles
NB = BS // 128  # 4 batch tiles per core
NO = OUT // 512  # 2 output column tiles
NDUMMY = 25

F32 = mybir.dt.float32
F16 = mybir.dt.float16
ALU = mybir.AluOpType
AF = mybir.ActivationFunctionType
AX = mybir.AxisListType


def _build_kernel(tc, out, xt, xn, wf, repeat=1):
    nc = tc.nc
    from contextlib import ExitStack

    octx = ExitStack()
    cpool = octx.enter_context(tc.tile_pool(name="const", bufs=1))
    zz = cpool.tile([128, 128], F16)
    hb = cpool.tile([128, BS], F16)  # 0.5 broadcast (for G1 = (x/2)*u)
    sb = cpool.tile([128, BS], F16)  # per-column s   (u = x*s + t)
    tb = cpool.tile([128, BS], F16)  # per-column t
    s_row = cpool.tile([1, BS], F16)
    t_row = cpool.tile([1, BS], F16)

    with (
        tc.tile_pool(name="psum_acc", bufs=1, space="PSUM") as pacc,
        tc.tile_pool(name="wa", bufs=1) as wapool,
        tc.tile_pool(name="wb", bufs=3) as wbpool,
        tc.tile_pool(name="xta", bufs=1) as xtpool,
        tc.tile_pool(name="xna", bufs=1) as xnpool,
        tc.tile_pool(name="g", bufs=2) as gpool,
        tc.tile_pool(name="u", bufs=2) as upool,
        tc.tile_pool(name="silu", bufs=1) as slpool,
        tc.tile_pool(name="o", bufs=2) as opool,
        tc.tile_pool(name="stats", bufs=1) as spool,
    ):
        po = [
            [
                pacc.tile([128, 512], F32, tag=f"po{t}{j}", name=f"po{t}{j}")
                for j in range(NO)
            ]
            for t in range(NB)
        ]
        xta = xtpool.tile([128, KT * BS], F16, tag="xta", name="xta")
        xna = xnpool.tile([128, NB * IN], F16, tag="xna", name="xna")

        def xk(k):  # [128, BS] fp16 block of x^T for contraction tile k
            return xta[:, k * BS : (k + 1) * BS]

        for rep in range(repeat):
            first = rep == 0
            if first:
                # tiny self-contained matmuls keep the PE busy (P-state
                # ramp) while the first input DMAs land
                nc.vector.memset(zz[:], 0.0)
                nc.vector.memset(hb[:], 0.5)
                for _ in range(NDUMMY):
                    nc.tensor.matmul(
                        po[0][0][:, 0:128], lhsT=zz[:], rhs=zz[:],
                        start=True, stop=True,
                    )

            # ---- input + phase-1 weight DMA stream (SP queue) ----
            # order tuned so the PE never starves: x block k0, wa0, x rest
            # (split), then wa_k interleaved with the stats loads
            was = [
                wapool.tile([128, 2 * OUT], F16, tag=f"wa{k}", name=f"wa{k}")
                for k in range(KT)
            ]

            def dma_wa(k):
                nc.sync.dma_start(
                    out=was[k][:], in_=wf[k * 128 : (k + 1) * 128, 0 : 2 * OUT]
                )

            def dma_xn(t):
                nc.sync.dma_start(
                    out=xna[:, t * IN : (t + 1) * IN],
                    in_=xn[:, t * IN : (t + 1) * IN],
                )

            if first:
                nc.sync.dma_start(out=xta[:, 0:BS], in_=xt[:, 0:BS])
                dma_wa(0)
                nc.sync.dma_start(out=xta[:, BS : 4 * BS], in_=xt[:, BS : 4 * BS])
                dma_wa(1)
                nc.sync.dma_start(out=xta[:, 4 * BS :], in_=xt[:, 4 * BS :])
                dma_xn(0)
                dma_wa(2)
                dma_xn(1)
                dma_wa(3)
                dma_xn(2)
                dma_wa(4)
                dma_xn(3)
                for k in range(5, KT):
                    dma_wa(k)
            else:
                for k in range(KT):
                    dma_wa(k)

            # ---- row stats -> sb/tb broadcast tiles (no PE, no PSUM) ----
            if first:
                for t in range(NB):
                    xnt = xna[:, t * IN : (t + 1) * IN]
                    mx = spool.tile([128, 1], F32, tag=f"mx{t}", name="mx")
                    mn = spool.tile([128, 1], F32, tag=f"mn{t}", name="mn")
                    nc.vector.tensor_reduce(mx[:], xnt, axis=AX.X, op=ALU.max)
                    nc.vector.tensor_reduce(mn[:], xnt, axis=AX.X, op=ALU.min)
                    d = spool.tile([128, 1], F32, tag=f"d{t}", name="d")
                    nc.vector.tensor_tensor(d[:], mx[:], mn[:], ALU.subtract)
                    r = spool.tile([128, 1], F32, tag=f"r{t}", name="r")
                    nc.vector.reciprocal(r[:], d[:])
                    sc = spool.tile([128, 1], F16, tag=f"sc{t}", name="sc")
                    nc.vector.tensor_scalar(sc[:], r[:], 4.0, None, ALU.mult)
                    tmp = spool.tile([128, 1], F32, tag=f"tm{t}", name="tm")
                    nc.vector.tensor_tensor(tmp[:], mn[:], sc[:], ALU.mult)
                    tcn = spool.tile([128, 1], F16, tag=f"tc{t}", name="tc")
                    nc.vector.tensor_scalar(
                        tcn[:], tmp[:], -1.0, -2.0, ALU.mult, ALU.add
                    )
                    tsl = slice(t * 128, (t + 1) * 128)
                    # strided SBUF->SBUF gathers: [128,1] column -> row slice
                    nc.scalar.dma_start(out=s_row[0:1, tsl], in_=sc[:, 0:1])
                    nc.scalar.dma_start(out=t_row[0:1, tsl], in_=tcn[:, 0:1])
                nc.gpsimd.partition_broadcast(sb[:], s_row[0:1, :])
                nc.gpsimd.partition_broadcast(tb[:], t_row[0:1, :])

            # ---- silu = x * sigmoid(x): sigmoid on ACT, multiply on DVE ----
            sls = []
            for k in range(KT):
                sg = slpool.tile([128, BS], F16, tag=f"sg{k}", name=f"sg{k}")
                sl = slpool.tile([128, BS], F16, tag=f"sl{k}", name=f"sl{k}")
                sls.append(sl)
                nc.scalar.activation(sg[:], xk(k), AF.Sigmoid)
                nc.vector.tensor_tensor(sl[:], sg[:], xk(k), ALU.mult)

            # ---- phase 1: d0 + silu matmuls (stats-independent) ----
            for k in range(KT):
                wa = was[k]
                for t in range(NB):
                    for j in range(NO):
                        nc.tensor.matmul(
                            po[t][j][:],
                            lhsT=xk(k)[:, t * 128 : (t + 1) * 128],
                            rhs=wa[:, OUT + j * 512 : OUT + (j + 1) * 512],
                            start=(k == 0),
                            stop=False,
                        )
                for t in range(NB):
                    for j in range(NO):
                        nc.tensor.matmul(
                            po[t][j][:],
                            lhsT=sls[k][:, t * 128 : (t + 1) * 128],
                            rhs=wa[:, j * 512 : (j + 1) * 512],
                            start=False,
                            stop=False,
                        )

            # ---- phase 2: chebyshev chain + d1..7 matmuls ----
            for k in range(KT):
                wb = wbpool.tile(
                    [128, (DEG - 1) * OUT], F16, tag="wbt", name="wbt"
                )
                nc.sync.dma_start(
                    out=wb[:],
                    in_=wf[k * 128 : (k + 1) * 128, 2 * OUT : (DEG + 1) * OUT],
                )
                gall = gpool.tile([128, (DEG - 1) * BS], F16, tag="gall", name="gall")

                def G(i):  # fp16 chain slots G_1..G_7
                    return gall[:, (i - 1) * BS : i * BS]

                ut = upool.tile([128, BS], F16, tag="ut", name="ut")
                xh = upool.tile([128, BS], F16, tag="xh", name="xh")
                nc.vector.tensor_tensor(ut[:], xk(k), sb[:], ALU.mult)
                nc.vector.tensor_tensor(ut[:], ut[:], tb[:], ALU.add)
                nc.vector.tensor_tensor(xh[:], xk(k), hb[:], ALU.mult)
                nc.vector.tensor_tensor(G(1), xh[:], ut[:], ALU.mult)
                for dg in range(2, DEG):
                    tmpd = upool.tile([128, BS], F16, tag=f"td{dg}", name="td")
                    nc.vector.tensor_tensor(tmpd[:], ut[:], G(dg - 1), ALU.mult)
                    prev2 = xk(k) if dg == 2 else G(dg - 2)
                    nc.vector.tensor_tensor(G(dg), tmpd[:], prev2, ALU.subtract)

                last = k == KT - 1
                if not last:
                    for m in range(1, DEG):
                        for t in range(NB):
                            for j in range(NO):
                                nc.tensor.matmul(
                                    po[t][j][:],
                                    lhsT=G(m)[:, t * 128 : (t + 1) * 128],
                                    rhs=wb[:, (m - 1) * OUT + j * 512 :
                                           (m - 1) * OUT + (j + 1) * 512],
                                    start=False,
                                    stop=False,
                                )
                else:
                    # bank-major: stop + drain each PSUM bank under the
                    # remaining matmuls; alternate ACT/DVE so the drains
                    # don't serialize on one queue
                    for bank, (t, j) in enumerate(
                        (t, j) for t in range(NB) for j in range(NO)
                    ):
                        for m in range(1, DEG):
                            nc.tensor.matmul(
                                po[t][j][:],
                                lhsT=G(m)[:, t * 128 : (t + 1) * 128],
                                rhs=wb[:, (m - 1) * OUT + j * 512 :
                                       (m - 1) * OUT + (j + 1) * 512],
                                start=False,
                                stop=(m == DEG - 1),
                            )
                        ot = opool.tile(
                            [128, 512], F16, tag=f"ot{bank % 4}", name="ot"
                        )
                        if bank % 2 == 0:
                            nc.scalar.activation(ot[:], po[t][j][:], AF.Copy)
                            dma_eng = nc.scalar
                        else:
                            nc.vector.tensor_copy(ot[:], po[t][j][:])
                            dma_eng = nc.sync
                        dma_eng.dma_start(
                            out=out[t * 128 : (t + 1) * 128,
                                    j * 512 : (j + 1) * 512],
                            in_=ot[:],
                        )
    octx.close()


_NC_CACHE = {}


def build_nc(repeat=1):
    if repeat in _NC_CACHE:
        return _NC_CACHE[repeat]
    nc = bacc.Bacc(
        "TRN2", target_bir_lowering=False, debug=False, num_devices=NCORES
    )
    xt = nc.dram_tensor("xt", [128, KT * BS], F16, kind="ExternalInput").ap()
    xn = nc.dram_tensor("xn", [128, NB * IN], F16, kind="ExternalInput").ap()
    wf = nc.dram_tensor("wf", [IN, (DEG + 1) * OUT], F16, kind="ExternalInput").ap()
    out = nc.dram_tensor("out", [BS, OUT], F16, kind="ExternalOutput").ap()
    with tile.TileContext(nc) as tc:
        _build_kernel(tc, out, xt, xn, wf, repeat=repeat)
    nc.compile()
    _NC_CACHE[repeat] = nc
    return nc


def make_in_maps(x, base_weight, cheb_weight):
    x = np.asarray(x, dtype=np.float32)
    wb = np.asarray(base_weight, dtype=np.float32)
    wc = np.asarray(cheb_weight, dtype=np.float32)
    # [wb | wc_d0 | .. | wc_d7] -> [IN, 9*OUT] fp16
    wf = np.concatenate(
        [wb[:, None, :], wc.transpose(0, 2, 1)], axis=1
    ).reshape(IN, (DEG + 1) * OUT).astype(np.float16)
    wf = np.ascontiguousarray(wf)
    in_maps = []
    for c in range(NCORES):
        shard = x[c * BS : (c + 1) * BS].astype(np.float16)  # [BS, IN]
        # xt: [128, KT*BS], block k = x^T rows k*128:(k+1)*128
        xt = np.ascontiguousarray(
            shard.T.reshape(KT, 128, BS).transpose(1, 0, 2).reshape(128, KT * BS)
        )
        # xn: [128, NB*IN], block t = rows t*128:(t+1)*128 of the shard
        xn = np.ascontiguousarray(
            shard.reshape(NB, 128, IN).transpose(1, 0, 2).reshape(128, NB * IN)
        )
        in_maps.append({"xt": xt, "xn": xn, "wf": wf})
    return in_maps


def kernel(x, base_weight, cheb_weight, degree=DEG, **_):
    assert int(degree) == DEG
    nc = build_nc()
    in_maps = make_in_maps(x, base_weight, cheb_weight)
    res = run_bass_kernel_spmd(nc, in_maps, list(range(NCORES)))
    return np.concatenate(
        [r["out"].astype(np.float32) for r in res.results], axis=0
    )
